# revision 4
# baseline (speedup 1.0000x reference)
"""Trainium2 Bass kernel for the recurrent Dense module.

Math (see reference):
    pre       = inputs @ W.T + b_vec            # [B, OUT]
    out       = pre + Aux[:,0] * state_vec      # [B, OUT]
    new_state = Aux[:,1] * state_vec + mean(pre, axis=0)   # [OUT]

Distribution: the batch (8192) is sharded over the 8 NeuronCores (1024 rows
each); W / b_vec / Aux / state_vec are replicated.  Each core computes its
[1024, OUT] slice of `out`; the host concatenates the slices.  `new_state`
depends on the batch only through mean(pre, 0) which by linearity equals
(mean_b inputs) @ W.T + b_vec, a [IN]x[IN,OUT] matvec the host computes
directly in float64.

On-chip layout per core: the matmul contracts over IN, so both operands are
staged with IN on the SBUF partition axis: lhsT = x_c.T (IN x 1024, the
stationary operand) and rhs = W.T (IN x OUT, the moving operand); both
transposes are done host-side during sharding.  PSUM tiles are
[128 batch, 512 out] fp32, accumulated over 8 k-tiles.  The affine shift
b_vec + Aux[:,0]*state_vec is folded in by the PSUM->SBUF eviction add.

Precision mode (BASS_DENSE_MODE env var, default bf16x3):
  bf16    1 matmul pass, operands rounded to bf16       (~2e-3 rel err)
  bf16x3  3 passes hi/hi hi/lo lo/hi of a bf16 split    (~5e-6 rel err)
  fp32    native fp32 matmuls (4 cycles/row)            (~1e-7 rel err)
  fp32r   float32r matmuls (1 cycle/row at N=512)
"""

import os

import ml_dtypes
import numpy as np

B, IN, OUT = 8192, 1024, 1024
N_CORES = 8
BS = B // N_CORES  # batch rows per core
P = 128  # SBUF partitions
NFREE = 512  # psum tile free dim (one bank of fp32)
KT = IN // P  # contraction tiles
MT = BS // P  # batch tiles per core
NT = OUT // NFREE  # out-feature tiles

MODE = os.environ.get("BASS_DENSE_MODE", "bf16x3")

_cache: dict = {}


def _build(mode):
    import concourse.bacc as bacc
    import concourse.mybir as mybir
    import concourse.tile as tile

    nc = bacc.Bacc()
    f32 = mybir.dt.float32
    if mode in ("bf16", "bf16x3"):
        dt_in = mybir.dt.bfloat16
    elif mode in ("fp16", "fp16x3"):
        dt_in = mybir.dt.float16
    elif mode == "fp32":
        dt_in = f32
    elif mode == "fp32r":
        dt_in = mybir.dt.float32r
    else:
        raise ValueError(mode)
    two_level = mode in ("bf16x3", "fp16x3")

    xh_d = nc.dram_tensor("xh", [IN, BS], dt_in, kind="ExternalInput")
    wh_d = nc.dram_tensor("wh", [IN, OUT], dt_in, kind="ExternalInput")
    if two_level:
        xl_d = nc.dram_tensor("xl", [IN, BS], dt_in, kind="ExternalInput")
        wl_d = nc.dram_tensor("wl", [IN, OUT], dt_in, kind="ExternalInput")
    shift_d = nc.dram_tensor("shift", [P, OUT], f32, kind="ExternalInput")
    out_d = nc.dram_tensor("out", [BS, OUT], f32, kind="ExternalOutput")

    with tile.TileContext(nc) as tc:
        with (
            tc.tile_pool(name="stat", bufs=1) as stat,
            tc.tile_pool(name="psum", bufs=8, space="PSUM") as pp,
            tc.tile_pool(name="evict", bufs=4) as ep,
        ):
            # Input loads in k order so the k-outer matmul loop can start as
            # soon as the first k-tile pair lands; shift is only needed at
            # eviction time so it loads last.
            xh_t, wh_t, xl_t, wl_t = [], [], [], []
            for k in range(KT):
                ks = slice(k * P, (k + 1) * P)
                t = stat.tile([P, BS], dt_in, tag=f"xh{k}", name=f"xh{k}")
                nc.sync.dma_start(out=t[:], in_=xh_d[ks, :])
                xh_t.append(t)
                t = stat.tile([P, OUT], dt_in, tag=f"wh{k}", name=f"wh{k}")
                nc.sync.dma_start(out=t[:], in_=wh_d[ks, :])
                wh_t.append(t)
                if two_level:
                    t = stat.tile([P, BS], dt_in, tag=f"xl{k}", name=f"xl{k}")
                    nc.sync.dma_start(out=t[:], in_=xl_d[ks, :])
                    xl_t.append(t)
                    t = stat.tile([P, OUT], dt_in, tag=f"wl{k}", name=f"wl{k}")
                    nc.sync.dma_start(out=t[:], in_=wl_d[ks, :])
                    wl_t.append(t)
            shift_t = stat.tile([P, OUT], f32, tag="shift")
            nc.sync.dma_start(out=shift_t[:], in_=shift_d[:])

            if two_level:
                passes = [(xh_t, wh_t), (xh_t, wl_t), (xl_t, wh_t)]
            else:
                passes = [(xh_t, wh_t)]
            n_pass = len(passes)

            # 16 output groups, processed in two rounds of 8 concurrent PSUM
            # banks, contraction (k) as the outer loop: the first matmuls only
            # need k-tile 0, so the PE starts ~2us in instead of waiting for
            # the whole operand load.
            groups = [(m, n) for m in range(MT) for n in range(NT)]
            NG = 8
            for g0 in range(0, len(groups), NG):
                batch = groups[g0 : g0 + NG]
                ps_t = [
                    pp.tile([P, NFREE], f32, tag="ps", name=f"ps{g0 + gi}")
                    for gi in range(len(batch))
                ]
                for k in range(KT):
                    for gi, (m, n) in enumerate(batch):
                        ms = slice(m * P, (m + 1) * P)
                        ns = slice(n * NFREE, (n + 1) * NFREE)
                        for pi, (xs, ws) in enumerate(passes):
                            nc.tensor.matmul(
                                ps_t[gi][:],
                                xs[k][:, ms],
                                ws[k][:, ns],
                                start=(k == 0 and pi == 0),
                                stop=(k == KT - 1 and pi == n_pass - 1),
                            )
                for gi, (m, n) in enumerate(batch):
                    ms = slice(m * P, (m + 1) * P)
                    ns = slice(n * NFREE, (n + 1) * NFREE)
                    ot = ep.tile([P, NFREE], f32, tag="ot", name=f"ot{g0 + gi}")
                    nc.vector.tensor_add(ot[:], ps_t[gi][:], shift_t[:, ns])
                    nc.sync.dma_start(out=out_d[ms, ns], in_=ot[:])

    nc.compile()
    return nc


def _get_nc(mode):
    if mode not in _cache:
        _cache[mode] = _build(mode)
    return _cache[mode]


def kernel(inputs, W, b_vec, Aux, state_vec, depth=None, _trace=False):
    from concourse.bass_utils import run_bass_kernel_spmd

    inputs = np.asarray(inputs, dtype=np.float32)
    W = np.asarray(W, dtype=np.float32)
    b_vec = np.asarray(b_vec, dtype=np.float32)
    Aux = np.asarray(Aux, dtype=np.float32)
    state_vec = np.asarray(state_vec, dtype=np.float32)

    mode = MODE
    nc = _get_nc(mode)

    lo_dt = {"bf16": ml_dtypes.bfloat16, "fp16": np.float16}.get(mode[:4])

    wt = np.ascontiguousarray(W.T)  # [IN, OUT]
    shift = (b_vec + Aux[:, 0] * state_vec).astype(np.float32)
    shift_rep = np.ascontiguousarray(np.broadcast_to(shift, (P, OUT)))

    if lo_dt is not None:
        wh = wt.astype(lo_dt)
        if mode.endswith("x3"):
            wl = (wt - wh.astype(np.float32)).astype(lo_dt)
    else:
        wh = wt

    in_maps = []
    for c in range(N_CORES):
        xt = np.ascontiguousarray(inputs[c * BS : (c + 1) * BS].T)  # [IN, BS]
        m = {"wh": wh, "shift": shift_rep}
        if lo_dt is not None:
            xh = xt.astype(lo_dt)
            m["xh"] = xh
            if mode.endswith("x3"):
                m["xl"] = (xt - xh.astype(np.float32)).astype(lo_dt)
                m["wl"] = wl
        else:
            m["xh"] = xt
        in_maps.append(m)

    res = run_bass_kernel_spmd(nc, in_maps, list(range(N_CORES)), trace=_trace)
    out = np.concatenate([res.results[c]["out"] for c in range(N_CORES)], axis=0)

    # new_state: by linearity mean(pre, 0) == (mean_b inputs) @ W.T + b_vec.
    col_mean = inputs.sum(axis=0, dtype=np.float64) / B  # [IN]
    mean_pre = col_mean @ W.T.astype(np.float64) + b_vec
    new_state = (Aux[:, 1].astype(np.float64) * state_vec + mean_pre).astype(
        np.float32
    )

    if _trace:
        return (out, new_state), res
    return out, new_state


# revision 8
# speedup vs baseline: 1.0299x; 1.0299x over previous
"""Trainium2 Bass kernel for the recurrent Dense module.

Math (see reference):
    pre       = inputs @ W.T + b_vec            # [B, OUT]
    out       = pre + Aux[:,0] * state_vec      # [B, OUT]
    new_state = Aux[:,1] * state_vec + mean(pre, axis=0)   # [OUT]

Distribution: the batch (8192) is sharded over the 8 NeuronCores (1024 rows
each); W / b_vec / Aux / state_vec are replicated.  Each core computes its
[1024, OUT] slice of `out`; the host concatenates the slices.  `new_state`
depends on the batch only through mean(pre, 0) which by linearity equals
(mean_b inputs) @ W.T + b_vec, a [IN]x[IN,OUT] matvec the host computes
directly in float64.

On-chip layout per core: the matmul contracts over IN, so both operands are
staged with IN on the SBUF partition axis: lhsT = x_c.T (IN x 1024, the
stationary operand) and rhs = W.T (IN x OUT, the moving operand); both
transposes are done host-side during sharding.  PSUM tiles are
[128 batch, 512 out] fp32, accumulated over 8 k-tiles.  The affine shift
b_vec + Aux[:,0]*state_vec is folded in by the PSUM->SBUF eviction add.

Precision mode (BASS_DENSE_MODE env var, default bf16x3):
  bf16    1 matmul pass, operands rounded to bf16       (~2e-3 rel err)
  bf16x3  3 passes hi/hi hi/lo lo/hi of a bf16 split    (~5e-6 rel err)
  fp32    native fp32 matmuls (4 cycles/row)            (~1e-7 rel err)
  fp32r   float32r matmuls (1 cycle/row at N=512)
"""

import os

import ml_dtypes
import numpy as np

B, IN, OUT = 8192, 1024, 1024
N_CORES = 8
BS = B // N_CORES  # batch rows per core
P = 128  # SBUF partitions
NFREE = 512  # psum tile free dim (one bank of fp32)
KT = IN // P  # contraction tiles
MT = BS // P  # batch tiles per core
NT = OUT // NFREE  # out-feature tiles

MODE = os.environ.get("BASS_DENSE_MODE", "bf16x3")

_cache: dict = {}


def _build(mode):
    import concourse.bacc as bacc
    import concourse.mybir as mybir
    import concourse.tile as tile

    nc = bacc.Bacc(enable_partition_id=False)
    f32 = mybir.dt.float32
    if mode in ("bf16", "bf16x3"):
        dt_in = mybir.dt.bfloat16
    elif mode in ("fp16", "fp16x3"):
        dt_in = mybir.dt.float16
    elif mode == "fp32":
        dt_in = f32
    elif mode == "fp32r":
        dt_in = mybir.dt.float32r
    else:
        raise ValueError(mode)
    two_level = mode in ("bf16x3", "fp16x3")

    xh_d = nc.dram_tensor("xh", [IN, BS], dt_in, kind="ExternalInput")
    wh_d = nc.dram_tensor("wh", [IN, OUT], dt_in, kind="ExternalInput")
    if two_level:
        xl_d = nc.dram_tensor("xl", [IN, BS], dt_in, kind="ExternalInput")
        wl_d = nc.dram_tensor("wl", [IN, OUT], dt_in, kind="ExternalInput")
    shift_d = nc.dram_tensor("shift", [P, OUT], f32, kind="ExternalInput")
    out_d = nc.dram_tensor("out", [BS, OUT], f32, kind="ExternalOutput")

    with tile.TileContext(nc) as tc:
        with (
            tc.tile_pool(name="stat", bufs=1) as stat,
            tc.tile_pool(name="psum", bufs=8, space="PSUM") as pp,
            tc.tile_pool(name="evict", bufs=4) as ep,
        ):
            # Input loads in k order so the k-outer matmul loop can start as
            # soon as the first k-tile pair lands; shift is only needed at
            # eviction time so it loads last.  DMA triggers cost ~600ns each
            # on the issuing engine, so round-robin them over four engines.
            trig = [nc.sync, nc.scalar, nc.gpsimd]
            ti = 0

            def dma(out, in_):
                nonlocal ti
                trig[ti % len(trig)].dma_start(out=out, in_=in_)
                ti += 1

            xh_t, wh_t, xl_t, wl_t = [], [], [], []
            for k in range(KT):
                ks = slice(k * P, (k + 1) * P)
                t = stat.tile([P, BS], dt_in, tag=f"xh{k}", name=f"xh{k}")
                dma(t[:], xh_d[ks, :])
                xh_t.append(t)
                t = stat.tile([P, OUT], dt_in, tag=f"wh{k}", name=f"wh{k}")
                dma(t[:], wh_d[ks, :])
                wh_t.append(t)
                if two_level:
                    t = stat.tile([P, BS], dt_in, tag=f"xl{k}", name=f"xl{k}")
                    dma(t[:], xl_d[ks, :])
                    xl_t.append(t)
                    t = stat.tile([P, OUT], dt_in, tag=f"wl{k}", name=f"wl{k}")
                    dma(t[:], wl_d[ks, :])
                    wl_t.append(t)
            shift_t = stat.tile([P, OUT], f32, tag="shift")
            dma(shift_t[:], shift_d[:])

            if two_level:
                passes = [(xh_t, wh_t), (xh_t, wl_t), (xl_t, wh_t)]
            else:
                passes = [(xh_t, wh_t)]
            n_pass = len(passes)

            # 16 output groups, processed in two rounds of 8 concurrent PSUM
            # banks, contraction (k) as the outer loop: the first matmuls only
            # need k-tile 0, so the PE starts ~2us in instead of waiting for
            # the whole operand load.
            groups = [(m, n) for m in range(MT) for n in range(NT)]
            NG = 8
            for g0 in range(0, len(groups), NG):
                batch = groups[g0 : g0 + NG]
                ps_t = [
                    pp.tile([P, NFREE], f32, tag="ps", name=f"ps{g0 + gi}")
                    for gi in range(len(batch))
                ]
                for k in range(KT):
                    for gi, (m, n) in enumerate(batch):
                        ms = slice(m * P, (m + 1) * P)
                        ns = slice(n * NFREE, (n + 1) * NFREE)
                        for pi, (xs, ws) in enumerate(passes):
                            nc.tensor.matmul(
                                ps_t[gi][:],
                                xs[k][:, ms],
                                ws[k][:, ns],
                                start=(k == 0 and pi == 0),
                                stop=(k == KT - 1 and pi == n_pass - 1),
                            )
                for gi, (m, n) in enumerate(batch):
                    ms = slice(m * P, (m + 1) * P)
                    ns = slice(n * NFREE, (n + 1) * NFREE)
                    ot = ep.tile([P, NFREE], f32, tag="ot", name=f"ot{g0 + gi}")
                    nc.vector.tensor_add(ot[:], ps_t[gi][:], shift_t[:, ns])
                    nc.scalar.dma_start(out=out_d[ms, ns], in_=ot[:])

    nc.compile()
    return nc


def _get_nc(mode):
    if mode not in _cache:
        _cache[mode] = _build(mode)
    return _cache[mode]


def kernel(inputs, W, b_vec, Aux, state_vec, depth=None, _trace=False):
    from concourse.bass_utils import run_bass_kernel_spmd

    inputs = np.asarray(inputs, dtype=np.float32)
    W = np.asarray(W, dtype=np.float32)
    b_vec = np.asarray(b_vec, dtype=np.float32)
    Aux = np.asarray(Aux, dtype=np.float32)
    state_vec = np.asarray(state_vec, dtype=np.float32)

    mode = MODE
    nc = _get_nc(mode)

    lo_dt = {"bf16": ml_dtypes.bfloat16, "fp16": np.float16}.get(mode[:4])

    wt = np.ascontiguousarray(W.T)  # [IN, OUT]
    shift = (b_vec + Aux[:, 0] * state_vec).astype(np.float32)
    shift_rep = np.ascontiguousarray(np.broadcast_to(shift, (P, OUT)))

    if lo_dt is not None:
        wh = wt.astype(lo_dt)
        if mode.endswith("x3"):
            wl = (wt - wh.astype(np.float32)).astype(lo_dt)
    else:
        wh = wt

    in_maps = []
    for c in range(N_CORES):
        xt = np.ascontiguousarray(inputs[c * BS : (c + 1) * BS].T)  # [IN, BS]
        m = {"wh": wh, "shift": shift_rep}
        if lo_dt is not None:
            xh = xt.astype(lo_dt)
            m["xh"] = xh
            if mode.endswith("x3"):
                m["xl"] = (xt - xh.astype(np.float32)).astype(lo_dt)
                m["wl"] = wl
        else:
            m["xh"] = xt
        in_maps.append(m)

    res = run_bass_kernel_spmd(nc, in_maps, list(range(N_CORES)), trace=_trace)
    out = np.concatenate([res.results[c]["out"] for c in range(N_CORES)], axis=0)

    # new_state: by linearity mean(pre, 0) == (mean_b inputs) @ W.T + b_vec.
    col_mean = inputs.sum(axis=0, dtype=np.float64) / B  # [IN]
    mean_pre = col_mean @ W.T.astype(np.float64) + b_vec
    new_state = (Aux[:, 1].astype(np.float64) * state_vec + mean_pre).astype(
        np.float32
    )

    if _trace:
        return (out, new_state), res
    return out, new_state


# revision 9
# speedup vs baseline: 1.1089x; 1.0768x over previous
"""Trainium2 Bass kernel for the recurrent Dense module.

Math (see reference):
    pre       = inputs @ W.T + b_vec            # [B, OUT]
    out       = pre + Aux[:,0] * state_vec      # [B, OUT]
    new_state = Aux[:,1] * state_vec + mean(pre, axis=0)   # [OUT]

Distribution: the batch (8192) is sharded over the 8 NeuronCores (1024 rows
each); W / b_vec / Aux / state_vec are replicated.  Each core computes its
[1024, OUT] slice of `out`; the host concatenates the slices.  `new_state`
depends on the batch only through mean(pre, 0) which by linearity equals
(mean_b inputs) @ W.T + b_vec, a [IN]x[IN,OUT] matvec the host computes
directly in float64.

On-chip layout per core: the matmul contracts over IN, so both operands are
staged with IN on the SBUF partition axis: lhsT = x_c.T (IN x 1024, the
stationary operand) and rhs = W.T (IN x OUT, the moving operand); both
transposes are done host-side during sharding.  PSUM tiles are
[128 batch, 512 out] fp32, accumulated over 8 k-tiles.  The affine shift
b_vec + Aux[:,0]*state_vec is folded in by the PSUM->SBUF eviction add.

Precision mode (BASS_DENSE_MODE env var, default bf16x3):
  bf16    1 matmul pass, operands rounded to bf16       (~2e-3 rel err)
  bf16x3  3 passes hi/hi hi/lo lo/hi of a bf16 split    (~5e-6 rel err)
  fp32    native fp32 matmuls (4 cycles/row)            (~1e-7 rel err)
  fp32r   float32r matmuls (1 cycle/row at N=512)
"""

import os

import ml_dtypes
import numpy as np

B, IN, OUT = 8192, 1024, 1024
N_CORES = 8
BS = B // N_CORES  # batch rows per core
P = 128  # SBUF partitions
NFREE = 512  # psum tile free dim (one bank of fp32)
KT = IN // P  # contraction tiles
MT = BS // P  # batch tiles per core
NT = OUT // NFREE  # out-feature tiles

MODE = os.environ.get("BASS_DENSE_MODE", "bf16x3")

_cache: dict = {}


def _build(mode):
    import concourse.bacc as bacc
    import concourse.mybir as mybir
    import concourse.tile as tile

    nc = bacc.Bacc(enable_partition_id=False)
    f32 = mybir.dt.float32
    if mode in ("bf16", "bf16x3"):
        dt_in = mybir.dt.bfloat16
    elif mode in ("fp16", "fp16x3"):
        dt_in = mybir.dt.float16
    elif mode == "fp32":
        dt_in = f32
    elif mode == "fp32r":
        dt_in = mybir.dt.float32r
    else:
        raise ValueError(mode)
    two_level = mode in ("bf16x3", "fp16x3")

    xh_d = nc.dram_tensor("xh", [IN, BS], dt_in, kind="ExternalInput")
    wh_d = nc.dram_tensor("wh", [IN, OUT], dt_in, kind="ExternalInput")
    if two_level:
        xl_d = nc.dram_tensor("xl", [IN, BS], dt_in, kind="ExternalInput")
        wl_d = nc.dram_tensor("wl", [IN, OUT], dt_in, kind="ExternalInput")
    shift_d = nc.dram_tensor("shift", [P, OUT], f32, kind="ExternalInput")
    out_d = nc.dram_tensor("out", [BS, OUT], f32, kind="ExternalOutput")

    with tile.TileContext(nc) as tc:
        with (
            tc.tile_pool(name="stat", bufs=1) as stat,
            tc.tile_pool(name="psum", bufs=8, space="PSUM") as pp,
            tc.tile_pool(name="evict", bufs=4) as ep,
        ):
            # Input loads in k order so the k-outer matmul loop can start as
            # soon as the first k-tile pair lands; shift is only needed at
            # eviction time so it loads last.  DMA triggers cost ~600ns each
            # on the issuing engine, so round-robin them over four engines.
            trig = [nc.sync, nc.scalar, nc.gpsimd]
            ti = 0

            def dma(out, in_):
                nonlocal ti
                trig[ti % len(trig)].dma_start(out=out, in_=in_)
                ti += 1

            xh_t, wh_t, xl_t, wl_t = [], [], [], []
            for k in range(KT):
                ks = slice(k * P, (k + 1) * P)
                t = stat.tile([P, BS], dt_in, tag=f"xh{k}", name=f"xh{k}")
                dma(t[:], xh_d[ks, :])
                xh_t.append(t)
                t = stat.tile([P, OUT], dt_in, tag=f"wh{k}", name=f"wh{k}")
                dma(t[:], wh_d[ks, :])
                wh_t.append(t)
                if two_level:
                    t = stat.tile([P, BS], dt_in, tag=f"xl{k}", name=f"xl{k}")
                    dma(t[:], xl_d[ks, :])
                    xl_t.append(t)
                    t = stat.tile([P, OUT], dt_in, tag=f"wl{k}", name=f"wl{k}")
                    dma(t[:], wl_d[ks, :])
                    wl_t.append(t)
            shift_t = stat.tile([P, OUT], f32, tag="shift")
            dma(shift_t[:], shift_d[:])

            if two_level:
                passes = [(xh_t, wh_t), (xh_t, wl_t), (xl_t, wh_t)]
            else:
                passes = [(xh_t, wh_t)]
            n_pass = len(passes)

            def evict(ps, m, n, gi):
                ms = slice(m * P, (m + 1) * P)
                ns = slice(n * NFREE, (n + 1) * NFREE)
                ot = ep.tile([P, NFREE], f32, tag="ot", name=f"ot{gi}")
                nc.vector.tensor_add(ot[:], ps[:], shift_t[:, ns])
                dma(out_d[ms, ns], ot[:])

            # 16 output groups.  Round 1 (7 groups) runs contraction-outer so
            # the PE starts as soon as k-tile 0 lands instead of waiting for
            # the whole operand load; its evictions overlap round 2.  Round 2
            # (9 groups, all operands resident by then) runs contraction-inner
            # so groups finish staggered and the eviction tail after the last
            # matmul is a single tile, not a pile of eight.
            groups = [(m, n) for m in range(MT) for n in range(NT)]
            R1 = 7
            ps1 = [
                pp.tile([P, NFREE], f32, tag="ps", name=f"ps{gi}")
                for gi in range(R1)
            ]
            for k in range(KT):
                for gi, (m, n) in enumerate(groups[:R1]):
                    ms = slice(m * P, (m + 1) * P)
                    ns = slice(n * NFREE, (n + 1) * NFREE)
                    for pi, (xs, ws) in enumerate(passes):
                        nc.tensor.matmul(
                            ps1[gi][:],
                            xs[k][:, ms],
                            ws[k][:, ns],
                            start=(k == 0 and pi == 0),
                            stop=(k == KT - 1 and pi == n_pass - 1),
                        )
            for gi, (m, n) in enumerate(groups[:R1]):
                evict(ps1[gi], m, n, gi)

            for gi, (m, n) in enumerate(groups[R1:], start=R1):
                ms = slice(m * P, (m + 1) * P)
                ns = slice(n * NFREE, (n + 1) * NFREE)
                ps = pp.tile([P, NFREE], f32, tag="ps", name=f"ps{gi}")
                idx = 0
                for k in range(KT):
                    for xs, ws in passes:
                        nc.tensor.matmul(
                            ps[:],
                            xs[k][:, ms],
                            ws[k][:, ns],
                            start=(idx == 0),
                            stop=(idx == KT * n_pass - 1),
                        )
                        idx += 1
                evict(ps, m, n, gi)

    nc.compile()
    return nc


def _get_nc(mode):
    if mode not in _cache:
        _cache[mode] = _build(mode)
    return _cache[mode]


def kernel(inputs, W, b_vec, Aux, state_vec, depth=None, _trace=False):
    from concourse.bass_utils import run_bass_kernel_spmd

    inputs = np.asarray(inputs, dtype=np.float32)
    W = np.asarray(W, dtype=np.float32)
    b_vec = np.asarray(b_vec, dtype=np.float32)
    Aux = np.asarray(Aux, dtype=np.float32)
    state_vec = np.asarray(state_vec, dtype=np.float32)

    mode = MODE
    nc = _get_nc(mode)

    lo_dt = {"bf16": ml_dtypes.bfloat16, "fp16": np.float16}.get(mode[:4])

    wt = np.ascontiguousarray(W.T)  # [IN, OUT]
    shift = (b_vec + Aux[:, 0] * state_vec).astype(np.float32)
    shift_rep = np.ascontiguousarray(np.broadcast_to(shift, (P, OUT)))

    if lo_dt is not None:
        wh = wt.astype(lo_dt)
        if mode.endswith("x3"):
            wl = (wt - wh.astype(np.float32)).astype(lo_dt)
    else:
        wh = wt

    in_maps = []
    for c in range(N_CORES):
        xt = np.ascontiguousarray(inputs[c * BS : (c + 1) * BS].T)  # [IN, BS]
        m = {"wh": wh, "shift": shift_rep}
        if lo_dt is not None:
            xh = xt.astype(lo_dt)
            m["xh"] = xh
            if mode.endswith("x3"):
                m["xl"] = (xt - xh.astype(np.float32)).astype(lo_dt)
                m["wl"] = wl
        else:
            m["xh"] = xt
        in_maps.append(m)

    res = run_bass_kernel_spmd(nc, in_maps, list(range(N_CORES)), trace=_trace)
    out = np.concatenate([res.results[c]["out"] for c in range(N_CORES)], axis=0)

    # new_state: by linearity mean(pre, 0) == (mean_b inputs) @ W.T + b_vec.
    col_mean = inputs.sum(axis=0, dtype=np.float64) / B  # [IN]
    mean_pre = col_mean @ W.T.astype(np.float64) + b_vec
    new_state = (Aux[:, 1].astype(np.float64) * state_vec + mean_pre).astype(
        np.float32
    )

    if _trace:
        return (out, new_state), res
    return out, new_state


# revision 11
# speedup vs baseline: 1.1132x; 1.0038x over previous
"""Trainium2 Bass kernel for the recurrent Dense module.

Math (see reference):
    pre       = inputs @ W.T + b_vec            # [B, OUT]
    out       = pre + Aux[:,0] * state_vec      # [B, OUT]
    new_state = Aux[:,1] * state_vec + mean(pre, axis=0)   # [OUT]

Distribution: the batch (8192) is sharded over the 8 NeuronCores (1024 rows
each); W / b_vec / Aux / state_vec are replicated.  Each core computes its
[1024, OUT] slice of `out`; the host concatenates the slices.  `new_state`
depends on the batch only through mean(pre, 0) which by linearity equals
(mean_b inputs) @ W.T + b_vec, a [IN]x[IN,OUT] matvec the host computes
directly in float64.

On-chip layout per core: the matmul contracts over IN, so both operands are
staged with IN on the SBUF partition axis: lhsT = x_c.T (IN x 1024, the
stationary operand) and rhs = W.T (IN x OUT, the moving operand); both
transposes are done host-side during sharding.  PSUM tiles are
[128 batch, 512 out] fp32, accumulated over 8 k-tiles.  The affine shift
b_vec + Aux[:,0]*state_vec is folded in by the PSUM->SBUF eviction add.

Precision mode (BASS_DENSE_MODE env var, default bf16x3):
  bf16    1 matmul pass, operands rounded to bf16       (~2e-3 rel err)
  bf16x3  3 passes hi/hi hi/lo lo/hi of a bf16 split    (~5e-6 rel err)
  fp32    native fp32 matmuls (4 cycles/row)            (~1e-7 rel err)
  fp32r   float32r matmuls (1 cycle/row at N=512)
"""

import os

import ml_dtypes
import numpy as np

B, IN, OUT = 8192, 1024, 1024
N_CORES = 8
BS = B // N_CORES  # batch rows per core
P = 128  # SBUF partitions
NFREE = 512  # psum tile free dim (one bank of fp32)
KT = IN // P  # contraction tiles
MT = BS // P  # batch tiles per core
NT = OUT // NFREE  # out-feature tiles

MODE = os.environ.get("BASS_DENSE_MODE", "bf16x3")

_cache: dict = {}


def _build(mode, with_shift=True):
    import concourse.bacc as bacc
    import concourse.mybir as mybir
    import concourse.tile as tile

    nc = bacc.Bacc(enable_partition_id=False)
    f32 = mybir.dt.float32
    if mode in ("bf16", "bf16x3"):
        dt_in = mybir.dt.bfloat16
    elif mode in ("fp16", "fp16x3"):
        dt_in = mybir.dt.float16
    elif mode == "fp32":
        dt_in = f32
    elif mode == "fp32r":
        dt_in = mybir.dt.float32r
    else:
        raise ValueError(mode)
    two_level = mode in ("bf16x3", "fp16x3")

    xh_d = nc.dram_tensor("xh", [IN, BS], dt_in, kind="ExternalInput")
    wh_d = nc.dram_tensor("wh", [IN, OUT], dt_in, kind="ExternalInput")
    if two_level:
        xl_d = nc.dram_tensor("xl", [IN, BS], dt_in, kind="ExternalInput")
        wl_d = nc.dram_tensor("wl", [IN, OUT], dt_in, kind="ExternalInput")
    if with_shift:
        shift_d = nc.dram_tensor("shift", [P, OUT], f32, kind="ExternalInput")
    out_d = nc.dram_tensor("out", [BS, OUT], f32, kind="ExternalOutput")

    with tile.TileContext(nc) as tc:
        with (
            tc.tile_pool(name="stat", bufs=1) as stat,
            tc.tile_pool(name="psum", bufs=8, space="PSUM") as pp,
            tc.tile_pool(name="evict", bufs=4) as ep,
        ):
            # Input loads in k order so the k-outer matmul loop can start as
            # soon as the first k-tile pair lands; shift is only needed at
            # eviction time so it loads last.  DMA triggers cost ~600ns each
            # on the issuing engine, so round-robin them over four engines.
            trig = [nc.sync, nc.scalar, nc.gpsimd]
            ti = 0

            def dma(out, in_):
                nonlocal ti
                trig[ti % len(trig)].dma_start(out=out, in_=in_)
                ti += 1

            xh_t, wh_t, xl_t, wl_t = [], [], [], []
            for k in range(KT):
                ks = slice(k * P, (k + 1) * P)
                t = stat.tile([P, BS], dt_in, tag=f"xh{k}", name=f"xh{k}")
                dma(t[:], xh_d[ks, :])
                xh_t.append(t)
                t = stat.tile([P, OUT], dt_in, tag=f"wh{k}", name=f"wh{k}")
                dma(t[:], wh_d[ks, :])
                wh_t.append(t)
                if two_level:
                    t = stat.tile([P, BS], dt_in, tag=f"xl{k}", name=f"xl{k}")
                    dma(t[:], xl_d[ks, :])
                    xl_t.append(t)
                    t = stat.tile([P, OUT], dt_in, tag=f"wl{k}", name=f"wl{k}")
                    dma(t[:], wl_d[ks, :])
                    wl_t.append(t)
            if with_shift:
                shift_t = stat.tile([P, OUT], f32, tag="shift")
                dma(shift_t[:], shift_d[:])

            if two_level:
                passes = [(xh_t, wh_t), (xh_t, wl_t), (xl_t, wh_t)]
            else:
                passes = [(xh_t, wh_t)]
            n_pass = len(passes)

            def evict(ps, m, n, gi):
                ms = slice(m * P, (m + 1) * P)
                ns = slice(n * NFREE, (n + 1) * NFREE)
                ot = ep.tile([P, NFREE], f32, tag="ot", name=f"ot{gi}")
                if with_shift:
                    nc.vector.tensor_add(ot[:], ps[:], shift_t[:, ns])
                else:
                    nc.vector.tensor_copy(ot[:], ps[:])
                dma(out_d[ms, ns], ot[:])

            # 16 output groups.  Round 1 (7 groups) runs contraction-outer so
            # the PE starts as soon as k-tile 0 lands instead of waiting for
            # the whole operand load; its evictions overlap round 2.  Round 2
            # (9 groups, all operands resident by then) runs contraction-inner
            # so groups finish staggered and the eviction tail after the last
            # matmul is a single tile, not a pile of eight.
            groups = [(m, n) for m in range(MT) for n in range(NT)]
            R1 = 7
            ps1 = [
                pp.tile([P, NFREE], f32, tag="ps", name=f"ps{gi}")
                for gi in range(R1)
            ]
            for k in range(KT):
                for gi, (m, n) in enumerate(groups[:R1]):
                    ms = slice(m * P, (m + 1) * P)
                    ns = slice(n * NFREE, (n + 1) * NFREE)
                    for pi, (xs, ws) in enumerate(passes):
                        nc.tensor.matmul(
                            ps1[gi][:],
                            xs[k][:, ms],
                            ws[k][:, ns],
                            start=(k == 0 and pi == 0),
                            stop=(k == KT - 1 and pi == n_pass - 1),
                        )
            for gi, (m, n) in enumerate(groups[:R1]):
                evict(ps1[gi], m, n, gi)

            for gi, (m, n) in enumerate(groups[R1:], start=R1):
                ms = slice(m * P, (m + 1) * P)
                ns = slice(n * NFREE, (n + 1) * NFREE)
                ps = pp.tile([P, NFREE], f32, tag="ps", name=f"ps{gi}")
                idx = 0
                for k in range(KT):
                    for xs, ws in passes:
                        nc.tensor.matmul(
                            ps[:],
                            xs[k][:, ms],
                            ws[k][:, ns],
                            start=(idx == 0),
                            stop=(idx == KT * n_pass - 1),
                        )
                        idx += 1
                evict(ps, m, n, gi)

    nc.compile()
    return nc


def _get_nc(mode, with_shift=True):
    key = (mode, with_shift)
    if key not in _cache:
        _cache[key] = _build(mode, with_shift)
    return _cache[key]


def kernel(inputs, W, b_vec, Aux, state_vec, depth=None, _trace=False):
    from concourse.bass_utils import run_bass_kernel_spmd

    inputs = np.asarray(inputs, dtype=np.float32)
    W = np.asarray(W, dtype=np.float32)
    b_vec = np.asarray(b_vec, dtype=np.float32)
    Aux = np.asarray(Aux, dtype=np.float32)
    state_vec = np.asarray(state_vec, dtype=np.float32)

    mode = MODE

    lo_dt = {"bf16": ml_dtypes.bfloat16, "fp16": np.float16}.get(mode[:4])

    wt = np.ascontiguousarray(W.T)  # [IN, OUT]
    shift = (b_vec + Aux[:, 0] * state_vec).astype(np.float32)
    with_shift = bool(np.any(shift))
    nc = _get_nc(mode, with_shift)

    if lo_dt is not None:
        wh = wt.astype(lo_dt)
        if mode.endswith("x3"):
            wl = (wt - wh.astype(np.float32)).astype(lo_dt)
    else:
        wh = wt

    in_maps = []
    for c in range(N_CORES):
        xt = np.ascontiguousarray(inputs[c * BS : (c + 1) * BS].T)  # [IN, BS]
        m = {"wh": wh}
        if with_shift:
            m["shift"] = np.ascontiguousarray(np.broadcast_to(shift, (P, OUT)))
        if lo_dt is not None:
            xh = xt.astype(lo_dt)
            m["xh"] = xh
            if mode.endswith("x3"):
                m["xl"] = (xt - xh.astype(np.float32)).astype(lo_dt)
                m["wl"] = wl
        else:
            m["xh"] = xt
        in_maps.append(m)

    res = run_bass_kernel_spmd(nc, in_maps, list(range(N_CORES)), trace=_trace)
    out = np.concatenate([res.results[c]["out"] for c in range(N_CORES)], axis=0)

    # new_state: by linearity mean(pre, 0) == (mean_b inputs) @ W.T + b_vec.
    col_mean = inputs.sum(axis=0, dtype=np.float64) / B  # [IN]
    mean_pre = col_mean @ W.T.astype(np.float64) + b_vec
    new_state = (Aux[:, 1].astype(np.float64) * state_vec + mean_pre).astype(
        np.float32
    )

    if _trace:
        return (out, new_state), res
    return out, new_state
